# revision 3
# baseline (speedup 1.0000x reference)
"""Trainium2 Bass kernel for nn_MHA_75110388072824.

Multi-head attention, B=2, T=2048, D=2048, NH=16 heads (hd=128), fp32,
causal mask, y = softmax(mask((x Wq^T)(x Wk^T)^T / sqrt(hd))) (x Wv^T) Wo^T.

Sharding over 8 NeuronCores: core = b*4 + hg, b in {0,1} batch,
hg in {0..3} a group of 4 heads (tensor-parallel columns of Wq/Wk/Wv,
rows of Wo).  Each core computes a partial output [T, D] = Z_hg @ Wo_hg^T;
the host sums the 4 head-group partials per batch.

Per-core pipeline (all matmuls in float32r = full-speed TF32-like):
  Phase A: stream x^T by 512-column chunks, weights resident; compute
           Q^T, K^T (per head, [hd=128 part, T]) and V ([s part, d]) and
           spill to DRAM scratch.
  Phase B: per head: reload Q^T/K^T (one DMA) + V slice; per 512-wide
           t-chunk: S = Q^T-block^T @ K^T (causal chunks only), additive
           mask on diagonal chunks, exp on ACT with fused row-sum
           (accum_out), normalize rows by 1/l on ACT, PE-transpose P
           blocks into [s, t] strips, PV matmuls accumulate Z^T.
  Phase C: out = Z^T^T @ Wo_hg^T via 4 head k-tiles, evicted to DRAM.
"""
import numpy as np

import concourse.bass as bass
import concourse.mybir as mybir
import concourse.tile as tile
from concourse import bacc
from concourse.bass_utils import run_bass_kernel_spmd

P = 128
T = 2048
D = 2048
NH = 16
HPC = 4            # heads per core
HD = 128
NT = T // P        # 16 t-blocks
NC4 = T // 512     # 4 512-chunks
KT = D // P        # 16 k-tiles over D
SCALE = 1.0 / float(np.sqrt(HD))
NEG = np.float32(-3.0e38)
F32 = mybir.dt.float32
F32R = mybir.dt.float32r
EXP = mybir.ActivationFunctionType.Exp
ADD = mybir.AluOpType.add
AXX = mybir.AxisListType.X


def _emit(tc, nc, xT, wqT, wkT, wvT, woT, masks, ident, out, qk_scr, v_scr,
          zt_scr):
    if True:
        # ---------------- Phase A: QKV projections -> DRAM scratch
        with (
            tc.tile_pool(name="wqkv", bufs=1) as wpool,
            tc.tile_pool(name="xs", bufs=2) as xpool,
            tc.tile_pool(name="aev", bufs=4) as aev,
            tc.tile_pool(name="aps", bufs=4, space="PSUM") as aps,
        ):
            wq_s = wpool.tile([P, KT, 512], F32R, tag="wq")
            wk_s = wpool.tile([P, KT, 512], F32R, tag="wk")
            wv_s = wpool.tile([P, KT, 512], F32R, tag="wv")
            nc.sync.dma_start(wq_s[:], wqT.rearrange("(ko p) d -> p ko d", p=P))
            nc.sync.dma_start(wk_s[:], wkT.rearrange("(ko p) d -> p ko d", p=P))
            nc.sync.dma_start(wv_s[:], wvT.rearrange("(ko p) d -> p ko d", p=P))
            xTr = xT.rearrange("(ko p) t -> p ko t", p=P)

            for tci in range(NC4):
                xs = xpool.tile([P, KT, 512], F32R, tag="xs")
                nc.sync.dma_start(xs[:], xTr[:, :, 512 * tci:512 * (tci + 1)])
                for w_s, off in ((wq_s, 0), (wk_s, T)):
                    for h in range(HPC):
                        ps = aps.tile([P, 512], F32, tag="ps")
                        for k in range(KT):
                            nc.tensor.matmul(
                                ps[:], w_s[:, k, h * P:(h + 1) * P], xs[:, k, :],
                                start=(k == 0), stop=(k == KT - 1))
                        ev = aev.tile([P, 512], F32R, tag="ev")
                        nc.any.tensor_copy(ev[:], ps[:])
                        nc.sync.dma_start(
                            qk_scr[h, :, off + 512 * tci:off + 512 * (tci + 1)], ev[:])
                for sb in range(4):
                    st = 4 * tci + sb
                    ps = aps.tile([P, 512], F32, tag="ps")
                    for k in range(KT):
                        nc.tensor.matmul(
                            ps[:], xs[:, k, sb * P:(sb + 1) * P], wv_s[:, k, :],
                            start=(k == 0), stop=(k == KT - 1))
                    ev = aev.tile([P, 512], F32R, tag="ev")
                    nc.any.tensor_copy(ev[:], ps[:])
                    nc.sync.dma_start(v_scr[st], ev[:])

        # ---------------- Phase B: attention per head
        with (
            tc.tile_pool(name="const", bufs=1) as cpool,
            tc.tile_pool(name="qk", bufs=2) as qkpool,
            tc.tile_pool(name="vh", bufs=2) as vhpool,
            tc.tile_pool(name="prow", bufs=6) as ppool,
            tc.tile_pool(name="pts", bufs=3) as ptspool,
            tc.tile_pool(name="lp", bufs=10) as lpool,
            tc.tile_pool(name="bps", bufs=3, space="PSUM") as bps,
            tc.tile_pool(name="tps", bufs=2, space="PSUM") as tps,
            tc.tile_pool(name="ztev", bufs=3) as ztev,
            tc.tile_pool(name="zps", bufs=2, space="PSUM") as zps,
        ):
            mask_t = cpool.tile([P, 4, 512], F32)
            nc.sync.dma_start(mask_t[:], masks)
            id_t = cpool.tile([P, P], F32)
            nc.sync.dma_start(id_t[:], ident)
            v_scr_r = v_scr.rearrange("a p d -> p a d")
            for h in range(HPC):
                qk = qkpool.tile([P, 2 * T], F32R, tag="qk")
                nc.sync.dma_start(qk[:], qk_scr[h])
                vh = vhpool.tile([P, NT, P], F32R, tag="vh")
                nc.sync.dma_start(vh[:], v_scr_r[:, :, h * P:(h + 1) * P])

                for tc2 in range(4):
                    nch = tc2 + 1           # causal 512-chunks for this row band
                    prows = []
                    for tj in range(4):
                        ti = 4 * tc2 + tj
                        prow = ppool.tile([P, T], F32, tag="prow")
                        lp = lpool.tile([P, 8], F32, tag="lp")
                        for c in range(nch):
                            sp = bps.tile([P, 512], F32, tag="sp")
                            nc.tensor.matmul(
                                sp[:], qk[:, ti * P:(ti + 1) * P],
                                qk[:, T + 512 * c:T + 512 * (c + 1)],
                                start=True, stop=True)
                            if c == tc2:
                                nc.vector.tensor_tensor(
                                    sp[:], sp[:], mask_t[:, tj, :], ADD)
                            nc.scalar.activation(
                                prow[:, 512 * c:512 * (c + 1)], sp[:], EXP,
                                bias=0.0, scale=SCALE, accum_out=lp[:, c:c + 1])
                        nc.vector.reduce_sum(lp[:, 4:5], lp[:, 0:nch], axis=AXX)
                        nc.vector.reciprocal(lp[:, 5:6], lp[:, 4:5])
                        nc.scalar.mul(prow[:, :512 * nch], prow[:, :512 * nch],
                                      lp[:, 5:6])
                        prows.append(prow)

                    ztp = zps.tile([P, 512], F32, tag="ztp")
                    ns = 4 * tc2 + 4
                    for si in range(ns):
                        ptp = tps.tile([P, 512], F32, tag="ptp")
                        for tj in range(4):
                            nc.tensor.matmul(
                                ptp[:, tj * P:(tj + 1) * P],
                                prows[tj][:, si * P:(si + 1) * P], id_t[:],
                                is_transpose=True, start=True, stop=True)
                        pts = ptspool.tile([P, 512], F32R, tag="pts")
                        nc.vector.tensor_copy(pts[:], ptp[:])
                        nc.tensor.matmul(ztp[:], vh[:, si, :], pts[:],
                                         start=(si == 0), stop=(si == ns - 1))
                    zev = ztev.tile([P, 512], F32R, tag="zev")
                    nc.any.tensor_copy(zev[:], ztp[:])
                    nc.sync.dma_start(
                        zt_scr[h, :, 512 * tc2:512 * (tc2 + 1)], zev[:])

        # ---------------- Phase C: output projection
        with (
            tc.tile_pool(name="wo", bufs=1) as wopool,
            tc.tile_pool(name="ztl", bufs=1) as ztlpool,
            tc.tile_pool(name="cev", bufs=4) as cev,
            tc.tile_pool(name="cps", bufs=4, space="PSUM") as cps,
        ):
            wo_s = wopool.tile([P, HPC, T], F32R)
            nc.sync.dma_start(wo_s[:], woT.rearrange("(ko p) d -> p ko d", p=P))
            zt_all = ztlpool.tile([P, HPC, T], F32R)
            nc.sync.dma_start(zt_all[:], zt_scr.rearrange("h p t -> p h t"))
            for ti in range(NT):
                for oc in range(4):
                    ps = cps.tile([P, 512], F32, tag="cps")
                    for h in range(HPC):
                        nc.tensor.matmul(
                            ps[:], zt_all[:, h, ti * P:(ti + 1) * P],
                            wo_s[:, h, 512 * oc:512 * (oc + 1)],
                            start=(h == 0), stop=(h == HPC - 1))
                    ev = cev.tile([P, 512], F32, tag="cev")
                    nc.any.tensor_copy(ev[:], ps[:])
                    nc.sync.dma_start(
                        out[ti * P:(ti + 1) * P, 512 * oc:512 * (oc + 1)], ev[:])


def build(repeat=1):
    nc = bacc.Bacc("TRN2", target_bir_lowering=False, debug=False)
    xT = nc.dram_tensor("xT", [D, T], F32R, kind="ExternalInput").ap()
    wqT = nc.dram_tensor("wqT", [D, 512], F32R, kind="ExternalInput").ap()
    wkT = nc.dram_tensor("wkT", [D, 512], F32R, kind="ExternalInput").ap()
    wvT = nc.dram_tensor("wvT", [D, 512], F32R, kind="ExternalInput").ap()
    woT = nc.dram_tensor("woT", [512, D], F32R, kind="ExternalInput").ap()
    masks = nc.dram_tensor("masks", [P, 4, 512], F32, kind="ExternalInput").ap()
    ident = nc.dram_tensor("ident", [P, P], F32, kind="ExternalInput").ap()
    out = nc.dram_tensor("out", [T, D], F32, kind="ExternalOutput").ap()
    qk_scr = nc.dram_tensor("qk_scr", [HPC, P, 2 * T], F32R).ap()
    v_scr = nc.dram_tensor("v_scr", [NT, P, 512], F32R).ap()
    zt_scr = nc.dram_tensor("zt_scr", [HPC, P, T], F32R).ap()

    with tile.TileContext(nc) as tc:
        if repeat == 1:
            _emit(tc, nc, xT, wqT, wkT, wvT, woT, masks, ident, out,
                  qk_scr, v_scr, zt_scr)
        else:
            with tc.For_i(0, repeat, 1):
                _emit(tc, nc, xT, wqT, wkT, wvT, woT, masks, ident, out,
                      qk_scr, v_scr, zt_scr)
    nc.compile()
    return nc


def make_inputs(x, Wq, Wk, Wv, Wo):
    """Host-side sharding: returns in_maps for cores 0..7 (core = b*4 + hg)."""
    masks = np.full((P, 4, 512), NEG, dtype=np.float32)
    for p in range(4):
        for t in range(P):
            masks[t, p, :128 * p + t + 1] = 0.0
    ident = np.eye(P, dtype=np.float32)
    xTs = [np.ascontiguousarray(x[b].T).astype(np.float32) for b in range(2)]
    in_maps = []
    for core in range(8):
        b, hg = core // 4, core % 4
        sl = slice(hg * 512, (hg + 1) * 512)
        in_maps.append({
            "xT": xTs[b],
            "wqT": np.ascontiguousarray(Wq[sl, :].T),
            "wkT": np.ascontiguousarray(Wk[sl, :].T),
            "wvT": np.ascontiguousarray(Wv[sl, :].T),
            "woT": np.ascontiguousarray(Wo[:, sl].T),
            "masks": masks,
            "ident": ident,
        })
    return in_maps


_nc_cache = {}


def kernel(x, Wq, Wk, Wv, Wo):
    x = np.asarray(x, dtype=np.float32)
    Wq = np.asarray(Wq, dtype=np.float32)
    Wk = np.asarray(Wk, dtype=np.float32)
    Wv = np.asarray(Wv, dtype=np.float32)
    Wo = np.asarray(Wo, dtype=np.float32)
    if "nc" not in _nc_cache:
        _nc_cache["nc"] = build()
    nc = _nc_cache["nc"]
    in_maps = make_inputs(x, Wq, Wk, Wv, Wo)
    res = run_bass_kernel_spmd(nc, in_maps, core_ids=list(range(8)))
    B = x.shape[0]
    out = np.zeros((B, T, D), dtype=np.float32)
    for core in range(8):
        b = core // 4
        out[b] += res.results[core]["out"]
    return out
